# revision 25
# baseline (speedup 1.0000x reference)
"""Trainium2 Bass kernel: causal multi-head attention (B=2, N=2048, C=2048, 16 heads).

Sharding: 16 heads split across 8 cores (2 heads/core, tensor parallel).
Each core computes q/k/v projections for its 2 heads, causal attention,
and its partial out-projection y_c = ctx_c @ wo_c.T. Host sums partials + bo.

All matmul operands are bf16 (PE streams 1 row/cycle; f32 PSUM accumulate).
Per-core layout:
  qT/kT: [head_dim(128) partitions, tokens]  (from lhsT=w^T, rhs=x^T)
  v^T is moved to V natural [tok, d] via one 3D-out DMA-XBAR transpose
  S^T[k, q] = K^T.T @ Q^T tiles              (contraction over head_dim)
  E^T = exp(scale * S^T) (no max subtraction -- scores are ~N(0, 1/9))
  row sums: E tiles accumulated over k-tiles, split by parity across DVE
  (f32r partial, feeds the reduce) and GpSimd (f32 partial); a single
  ones-matmul per (b,qc,h) reduces across partitions and broadcasts;
  wide reciprocal + multiply normalizes ctx^T
  ctx^T[d, q] = V.T @ E^T  (lhsT = V natural [tok, d])
  y[tok, f] = ctx^T.T @ wo^T  (natural output layout, written as bf16)
Causality: only k-tiles with k <= q_max computed; within diagonal k-tiles
the S/exp/AV/esum work is narrowed to the live column range [128a, 512)
and a single [128,128] triangular mask handles the partial block.

Schedule: one fused stream. Segment ch = projection matmuls of chunk ch
(pure PE work) woven at instruction granularity with the attention
kt-chain of block ch-1 (ACT/DVE-heavy; exp throughput, not PE, limits
it) and the out-projection of block ch-2. This keeps every engine's
work spread across the whole kernel instead of phase-bunched, removes
the phase-boundary PSUM handoff, and fills the PE during exp waits.
"""

import os
import numpy as np
import ml_dtypes

import concourse.bass as bass
import concourse.tile as tile
from concourse import bacc, mybir
from concourse import bass_utils

F32 = mybir.dt.float32
F32R = mybir.dt.float32r
BF16 = mybir.dt.bfloat16
AF = mybir.ActivationFunctionType
ADD = mybir.AluOpType.add

# problem dims (hardcoded per contract)
B = 2
N = 2048
C = 2048
HEADS = 16
HD = 128          # head dim
NCORES = 8
HPC = HEADS // NCORES  # heads per core = 2
E = HPC * HD      # per-core projection width = 256
BN = B * N        # 4096
P = 128
CT = C // P       # 16 contraction tiles
NCH = 512         # n-chunk width for projections
NCHUNKS = BN // NCH   # 8
QCW = 512         # q-chunk width in attention
QCHUNKS = N // QCW    # 4 per batch
KT_PER_B = N // P     # 16 k-tiles per batch
TOK_TILES = BN // P   # 32
SCALE = float(HD) ** -0.5
XQ = 8            # x streamed in eighths of 2 c-tiles

_CACHE = {}


def _weave(*streams):
    """Emit step-closures from several streams, interleaved proportionally
    so all streams finish together. Each step is a 0-arg callable."""
    streams = [list(s) for s in streams if s]
    if not streams:
        return
    total = max(len(s) for s in streams)
    idx = [0.0] * len(streams)
    inc = [len(s) / total for s in streams]
    for _ in range(total):
        for i, s in enumerate(streams):
            idx[i] += inc[i]
            while idx[i] >= 1.0 and s:
                s.pop(0)()
                idx[i] -= 1.0
    for s in streams:
        for step in s:
            step()


def _build():
    nc = bacc.Bacc(
        "TRN2",
        target_bir_lowering=False,
        debug=False,
        enable_asserts=False,
        num_devices=NCORES,
    )

    xT = nc.dram_tensor("xT", [C, BN], BF16, kind="ExternalInput").ap()
    # wq/wk/wv packed per 2-c-tile piece: [piece, p, wi, t, e] so one DMA
    # brings the piece of all three projections
    wpk = nc.dram_tensor("wpk", [XQ, P, 3, 2, E], BF16,
                         kind="ExternalInput").ap()
    woT = nc.dram_tensor("woT", [E, C], BF16, kind="ExternalInput").ap()
    bqh = nc.dram_tensor("bqh", [HPC, P], F32, kind="ExternalInput").ap()
    bkh = nc.dram_tensor("bkh", [HPC, P], F32, kind="ExternalInput").ap()
    bvh = nc.dram_tensor("bvh", [HPC, P], F32, kind="ExternalInput").ap()
    masks = nc.dram_tensor("masks", [P, P], BF16, kind="ExternalInput").ap()
    ones_d = nc.dram_tensor("ones_d", [P, P], F32R, kind="ExternalInput").ap()
    yp = nc.dram_tensor("yp", [BN, C], BF16, kind="ExternalOutput").ap()

    with tile.TileContext(nc) as tc:
        with tc.tile_pool(name="persist", bufs=1) as persist, \
             tc.tile_pool(name="wpool", bufs=1) as wpool, \
             tc.tile_pool(name="xpool", bufs=24) as xpool, \
             tc.tile_pool(name="vtpool", bufs=2) as vtpool, \
             tc.tile_pool(name="epool", bufs=10) as epool, \
             tc.tile_pool(name="ctxpool", bufs=6) as ctxpool, \
             tc.tile_pool(name="espool", bufs=4) as espool, \
             tc.tile_pool(name="smpool", bufs=3) as smpool, \
             tc.tile_pool(name="ysbpool", bufs=3) as ysbpool, \
             tc.tile_pool(name="pacc_ps", bufs=2, space="PSUM") as pacc, \
             tc.tile_pool(name="sps_ps", bufs=2, space="PSUM") as spool, \
             tc.tile_pool(name="ctx_ps", bufs=2, space="PSUM") as cps, \
             tc.tile_pool(name="ysum_ps", bufs=2, space="PSUM") as yps:
            qT = persist.tile([P, HPC, B, N], BF16, tag="qT")
            kT = persist.tile([P, HPC, B, N], BF16, tag="kT")
            vN = persist.tile([P, TOK_TILES, E], BF16, tag="vN")
            masks_sb = persist.tile([P, P], BF16, tag="masks")
            ones_sb = persist.tile([P, P], F32R, tag="ones")

            wqkv_sb = wpool.tile([P, 3, CT, E], BF16, tag="wqkv")
            bq_sb = wpool.tile([P, HPC], F32, tag="bq")
            bk_sb = wpool.tile([P, HPC], F32, tag="bk")
            bv_sb = wpool.tile([P, HPC], F32, tag="bv")
            wo_sb = wpool.tile([P, HPC, C], BF16, tag="wo")

            wbias = [(0, bq_sb, bqh), (1, bk_sb, bkh), (2, bv_sb, bvh)]
            xTr = xT.rearrange("(t p) n -> p t n", p=P)

            def dma_x_chunk(ch):
                """Prefetch the 8 pieces of x chunk ch."""
                n0 = ch * NCH
                xh = []
                for piece in range(XQ):
                    xc = xpool.tile([P, CT // XQ, NCH], BF16, tag="xc",
                                    name=f"xc_{ch}_{piece}")
                    nc.sync.dma_start(
                        xc[:], xTr[:, piece * 2:(piece + 1) * 2,
                                   n0:n0 + NCH])
                    xh.append(xc)
                return xh

            # PE warm-up: memset a dummy tile and run throwaway matmuls so
            # the tensor engine leaves its low p-state while the first x/w
            # pieces are still in flight.
            warm = wpool.tile([P, P], BF16, tag="warm")
            warm_ps = pacc.tile([P, P], F32, tag="pacc", name="warm_ps")
            nc.gpsimd.memset(warm[:], 0.0)
            for _ in range(16):
                nc.tensor.matmul(warm_ps[:], warm[:], warm[:],
                                 start=True, stop=True)

            # Initial DMA priority: weight pieces on the SP queue, chunk-0
            # and chunk-1 x pieces on the (idle at startup) scalar queue —
            # two queues issue in parallel so the PE isn't starved during
            # the first chunk. Steady-state prefetch reverts to SP.
            xh0 = []
            xh1 = []
            wsbr = wqkv_sb[:].rearrange("p w (pc t) e -> p pc w t e", t=2)
            for piece in range(8):
                nc.sync.dma_start(wsbr[:, piece], wpk[piece])
                xc = xpool.tile([P, CT // XQ, NCH], BF16, tag="xc",
                                name=f"xc_0_{piece}")
                nc.scalar.dma_start(
                    xc[:], xTr[:, piece * 2:(piece + 1) * 2, 0:NCH])
                xh0.append(xc)
                if piece == 2:
                    for (_, bdst, bsrc) in wbias:
                        nc.sync.dma_start(
                            bdst[:], bsrc.rearrange("h p -> p h"))
                    nc.sync.dma_start(masks_sb[:], masks)
                    nc.sync.dma_start(ones_sb[:], ones_d)
            for piece in range(8):
                xc = xpool.tile([P, CT // XQ, NCH], BF16, tag="xc",
                                name=f"xc_1_{piece}")
                nc.scalar.dma_start(
                    xc[:], xTr[:, piece * 2:(piece + 1) * 2, NCH:2 * NCH])
                xh1.append(xc)
            nc.sync.dma_start(wo_sb[:], woT.rearrange("(h p) f -> p h f", p=P))

            # ---------------- step generators ----------------

            def proj_steps(ch, xh):
                """Projection of chunk ch: 6 runs (q/k/v x 2 heads) of 16
                accumulating matmuls, one PSUM bank at a time. Yields
                single-matmul closures; epilogue (bias copy / v transpose)
                rides with the last matmul of each run."""
                b = ch // (N // NCH)
                nn0 = (ch % (N // NCH)) * NCH
                steps = []
                for wi, bsb, _ in wbias:
                    for h in range(HPC):
                        state = {}

                        def mk(ct, bsb=bsb, wi=wi, h=h, state=state):
                            def step():
                                if ct == 0:
                                    state["acc"] = pacc.tile(
                                        [P, NCH], F32, tag="pacc",
                                        name=f"pacc_{ch}_{wi}_{h}")
                                nc.tensor.matmul(
                                    state["acc"][:],
                                    wqkv_sb[:, wi, ct, h * HD:(h + 1) * HD],
                                    xh[ct // 2][:, ct % 2, :],
                                    start=(ct == 0), stop=(ct == CT - 1),
                                )
                                if ct == CT - 1:
                                    acc = state["acc"]
                                    if wi < 2:
                                        dst = (qT if wi == 0 else kT)
                                        nc.scalar.activation(
                                            dst[:, h, b, nn0:nn0 + NCH],
                                            acc[:], AF.Identity,
                                            bias=bsb[:, h:h + 1], scale=1.0)
                                    else:
                                        vt = vtpool.tile(
                                            [P, NCH], BF16, tag="vt",
                                            name=f"vt_{ch}_{h}")
                                        nc.scalar.activation(
                                            vt[:], acc[:], AF.Identity,
                                            bias=bsb[:, h:h + 1], scale=1.0)
                                        nc.sync.dma_start(
                                            vN[:, ch * (NCH // P):
                                               (ch + 1) * (NCH // P),
                                               h * HD:(h + 1) * HD],
                                            vt[:], transpose=True)
                            return step
                        for ct in range(CT):
                            steps.append(mk(ct))
                return steps

            def attn_steps(b, qc, ctx_tiles):
                """Attention for block (b, qc): per (h, kt) one closure
                (S, exp, mask, esum, AV); per-h finalize closure appends
                the normalized ctx tile to ctx_tiles."""
                nkt = 4 * qc + 4
                steps = []
                for h in range(HPC):
                    state = {}

                    def emit_av(kt, state=state, h=h):
                        # AV for k-tile kt, one step behind its S/exp so
                        # the exp+mask chain is off the PE critical path
                        a = kt - 4 * qc
                        a0 = P * a if a > 0 else 0
                        nc.tensor.matmul(
                            state["ctxu"][:, a0:],
                            vN[:, b * KT_PER_B + kt,
                               h * HD:(h + 1) * HD],
                            state["et"][kt][:, a0:],
                            start=(kt == 0), stop=(kt == nkt - 1),
                        )

                    def mk(kt, h=h, state=state, emit_av=emit_av):
                        a = kt - 4 * qc
                        a0 = P * a if a > 0 else 0

                        def step():
                            if kt == 0:
                                state["ctxu"] = cps.tile(
                                    [P, QCW], F32, tag="ctxu",
                                    name=f"ctxu_{b}_{qc}_{h}")
                                state["esumA"] = espool.tile(
                                    [P, QCW], F32R, tag="esumA",
                                    name=f"esumA_{b}_{qc}_{h}")
                                state["esumB"] = espool.tile(
                                    [P, QCW], F32, tag="esumB",
                                    name=f"esumB_{b}_{qc}_{h}")
                                state["et"] = {}
                            esumA, esumB = state["esumA"], state["esumB"]
                            sps = spool.tile([P, QCW], F32, tag="s",
                                             name=f"s_{b}_{qc}_{h}_{kt}")
                            nc.tensor.matmul(
                                sps[:, a0:],
                                kT[:, h, b, kt * P:(kt + 1) * P],
                                qT[:, h, b, qc * QCW + a0:(qc + 1) * QCW],
                                start=True, stop=True,
                            )
                            et = epool.tile([P, QCW], BF16, tag="e",
                                            name=f"e_{b}_{qc}_{h}_{kt}")
                            state["et"][kt] = et
                            nc.scalar.activation(
                                et[:, a0:], sps[:, a0:], AF.Exp, scale=SCALE)
                            if a >= 0:  # diagonal: triangular [128,128]
                                nc.vector.tensor_mul(
                                    et[:, a0:a0 + P], et[:, a0:a0 + P],
                                    masks_sb[:])
                            if kt == 0:
                                nc.vector.tensor_copy(esumA[:], et[:])
                            elif kt == 1:
                                nc.gpsimd.tensor_copy(
                                    esumB[:, a0:], et[:, a0:])
                            elif kt % 2 == 0:
                                nc.vector.tensor_tensor(
                                    esumA[:, a0:], esumA[:, a0:],
                                    et[:, a0:], op=ADD)
                            else:
                                nc.gpsimd.tensor_tensor(
                                    esumB[:, a0:], esumB[:, a0:],
                                    et[:, a0:], op=ADD)
                            if kt >= 1:
                                emit_av(kt - 1)
                        return step

                    def fin(h=h, state=state, emit_av=emit_av):
                        def step():
                            emit_av(nkt - 1)
                            esumA, esumB = state["esumA"], state["esumB"]
                            c0 = P if qc == 0 else 0
                            nc.vector.tensor_tensor(
                                esumA[:, c0:], esumA[:, c0:], esumB[:, c0:],
                                op=ADD)
                            sums_bc = spool.tile([P, QCW], F32, tag="s",
                                                 name=f"sumbc_{b}_{qc}_{h}")
                            nc.tensor.matmul(
                                sums_bc[:], ones_sb[:], esumA[:],
                                start=True, stop=True)
                            recip_bc = smpool.tile([P, QCW], F32,
                                                   tag="recipbc",
                                                   name=f"recip_{b}_{qc}_{h}")
                            nc.vector.reciprocal_approx_fast(
                                recip_bc[:], sums_bc[:])
                            ctx = ctxpool.tile([P, QCW], BF16, tag="ctx",
                                               name=f"ctx_{b}_{qc}_{h}")
                            nc.vector.tensor_mul(
                                ctx[:], state["ctxu"][:], recip_bc[:])
                            ctx_tiles.append(ctx)
                        return step

                    for kt in range(nkt):
                        steps.append(mk(kt))
                    steps.append(fin())
                return steps

            def outproj_steps(b, qc, ctx_tiles):
                """Out-projection of block (b, qc): one closure per
                (nt, fc) = 2 accumulating matmuls + PSUM->SBUF copy; DMA
                per nt rides with the last fc."""
                steps = []
                for nt in range(QCW // P):
                    state = {}

                    def mk(fc, nt=nt, state=state):
                        def step():
                            if fc == 0:
                                state["ysb"] = ysbpool.tile(
                                    [P, C], BF16, tag="ysb",
                                    name=f"ysb_{b}_{qc}_{nt}")
                            y_ps = yps.tile([P, 512], F32, tag="yps",
                                            name=f"yps_{b}_{qc}_{nt}_{fc}")
                            for h in range(HPC):
                                nc.tensor.matmul(
                                    y_ps[:],
                                    ctx_tiles[h][:, nt * P:(nt + 1) * P],
                                    wo_sb[:, h, fc * 512:(fc + 1) * 512],
                                    start=(h == 0), stop=(h == HPC - 1),
                                )
                            y_sb = state["ysb"]
                            if fc % 2 == 0:
                                nc.vector.tensor_copy(
                                    y_sb[:, fc * 512:(fc + 1) * 512],
                                    y_ps[:])
                            else:
                                nc.scalar.copy(
                                    y_sb[:, fc * 512:(fc + 1) * 512],
                                    y_ps[:])
                            # write back per fc-pair so the final gate at
                            # kernel end is one small slice, not [128, 2048]
                            if fc % 2 == 1:
                                row0 = b * N + qc * QCW + nt * P
                                nc.sync.dma_start(
                                    yp[row0:row0 + P,
                                       (fc - 1) * 512:(fc + 1) * 512],
                                    y_sb[:, (fc - 1) * 512:(fc + 1) * 512])
                        return step
                    for fc in range(C // 512):
                        steps.append(mk(fc))
                return steps

            # ---------------- fused schedule ----------------
            # segment ch: proj(ch) ⊗ attn(block ch-1) ⊗ outproj(block ch-2)
            blocks = [(ch // QCHUNKS, ch % QCHUNKS) for ch in range(NCHUNKS)]
            ctxs = {}      # block index -> ctx tile list
            chunk_x = {0: xh0, 1: xh1}
            for ch in range(NCHUNKS):
                if ch + 2 < NCHUNKS:
                    chunk_x[ch + 2] = dma_x_chunk(ch + 2)
                ps = proj_steps(ch, chunk_x.pop(ch))
                streams = [ps]
                if ch >= 1:
                    bb, qq = blocks[ch - 1]
                    ctxs[ch - 1] = []
                    streams.append(attn_steps(bb, qq, ctxs[ch - 1]))
                if ch >= 2:
                    bb, qq = blocks[ch - 2]
                    streams.append(outproj_steps(bb, qq, ctxs[ch - 2]))
                _weave(*streams)
            # tail: attn of the last block ⊗ outproj of the second-to-last,
            # then the last block's outproj
            bb, qq = blocks[NCHUNKS - 1]
            ctxs[NCHUNKS - 1] = []
            _weave(attn_steps(bb, qq, ctxs[NCHUNKS - 1]),
                   outproj_steps(*blocks[NCHUNKS - 2], ctxs[NCHUNKS - 2]))
            for step in outproj_steps(bb, qq, ctxs[NCHUNKS - 1]):
                step()

    nc.compile()
    return nc


def _host_prep(x, wq, bq, wk, bk, wv, bv, wo):
    """Build the 8 per-core input maps."""
    x = np.asarray(x, dtype=np.float32)
    xT = np.ascontiguousarray(x.reshape(BN, C).T).astype(ml_dtypes.bfloat16)

    # triangular mask for the partial 128-col block of a diagonal k-tile
    kl = np.arange(P)[:, None]
    jj = np.arange(P)[None, :]
    m = (jj >= kl).astype(ml_dtypes.bfloat16)

    in_maps = []
    for c in range(NCORES):
        e0 = c * E
        # packed weights: wpk[piece, p, wi, t, e] = w{wi}[e0+e, (2piece+t)*P+p]
        ws = np.stack([np.asarray(wq)[e0:e0 + E, :].T,
                       np.asarray(wk)[e0:e0 + E, :].T,
                       np.asarray(wv)[e0:e0 + E, :].T])  # [3, C, E]
        wpk = np.ascontiguousarray(
            ws.reshape(3, XQ, 2, P, E).transpose(1, 3, 0, 2, 4)
        ).astype(ml_dtypes.bfloat16)
        in_maps.append({
            "xT": xT,
            "wpk": wpk,
            "woT": np.ascontiguousarray(np.asarray(wo)[:, e0:e0 + E].T).astype(ml_dtypes.bfloat16),
            "bqh": np.ascontiguousarray(
                np.asarray(bq)[e0:e0 + E].reshape(HPC, P)).astype(np.float32),
            "bkh": np.ascontiguousarray(
                np.asarray(bk)[e0:e0 + E].reshape(HPC, P)).astype(np.float32),
            "bvh": np.ascontiguousarray(
                np.asarray(bv)[e0:e0 + E].reshape(HPC, P)).astype(np.float32),
            "masks": m,
            "ones_d": np.ones((P, P), dtype=np.float32),
        })
    return in_maps


def _ensure_ntff_hook_module():
    """run_bass_kernel_spmd(trace=True) imports antenv.axon_hooks; provide a
    stub (hook=None -> tracing skipped gracefully) if the module is absent."""
    try:
        import antenv.axon_hooks  # noqa: F401
    except ImportError:
        import sys
        import types
        try:
            import antenv
        except ImportError:
            return
        mod = types.ModuleType("antenv.axon_hooks")
        state = {"hook": None}
        mod.set_axon_ntff_profile_hook = lambda h: state.__setitem__("hook", h)
        mod.get_axon_ntff_profile_hook = lambda: state["hook"]
        sys.modules["antenv.axon_hooks"] = mod
        antenv.axon_hooks = mod


def kernel(**inputs):
    _ensure_ntff_hook_module()
    if "nc" not in _CACHE:
        _CACHE["nc"] = _build()
    nc = _CACHE["nc"]

    in_maps = _host_prep(
        inputs["x"], inputs["wq"], inputs["bq"], inputs["wk"], inputs["bk"],
        inputs["wv"], inputs["bv"], inputs["wo"],
    )

    res = bass_utils.run_bass_kernel_spmd(
        nc, in_maps, core_ids=list(range(NCORES)),
        trace=bool(os.environ.get("BASS_TRACE")),
    )
    _CACHE["last_result"] = res

    y = np.zeros((BN, C), dtype=np.float64)
    for c in range(NCORES):
        y += res.results[c]["yp"].astype(np.float64)
    y += np.asarray(inputs["bo"], dtype=np.float64)
    return y.astype(np.float32).reshape(B, N, C)


# revision 29
# speedup vs baseline: 1.0213x; 1.0213x over previous
"""Trainium2 Bass kernel: causal multi-head attention (B=2, N=2048, C=2048, 16 heads).

Sharding: 16 heads split across 8 cores (2 heads/core, tensor parallel).
Each core computes q/k/v projections for its 2 heads, causal attention,
and its partial out-projection y_c = ctx_c @ wo_c.T. Host sums partials + bo.

All matmul operands are bf16 (PE streams 1 row/cycle; f32 PSUM accumulate).
Per-core layout:
  qT/kT: [head_dim(128) partitions, tokens]  (from lhsT=w^T, rhs=x^T)
  v^T is moved to V natural [tok, d] via one 3D-out DMA-XBAR transpose
  S^T[k, q] = K^T.T @ Q^T tiles              (contraction over head_dim)
  E^T = exp(scale * S^T) (no max subtraction -- scores are ~N(0, 1/9))
  row sums: E tiles accumulated over k-tiles, split by parity across DVE
  (f32r partial, feeds the reduce) and GpSimd (f32 partial); a single
  ones-matmul per (b,qc,h) reduces across partitions and broadcasts;
  wide reciprocal + multiply normalizes ctx^T
  ctx^T[d, q] = V.T @ E^T  (lhsT = V natural [tok, d])
  y[tok, f] = ctx^T.T @ wo^T  (natural output layout, written as bf16)
Causality: only k-tiles with k <= q_max computed; within diagonal k-tiles
the S/exp/AV/esum work is narrowed to the live column range [128a, 512)
and a single [128,128] triangular mask handles the partial block.

Schedule: one fused stream. Segment ch = projection matmuls of chunk ch
(pure PE work) woven at instruction granularity with the attention
kt-chain of block ch-1 (ACT/DVE-heavy; exp throughput, not PE, limits
it) and the out-projection of block ch-2. This keeps every engine's
work spread across the whole kernel instead of phase-bunched, removes
the phase-boundary PSUM handoff, and fills the PE during exp waits.
"""

import os
import numpy as np
import ml_dtypes

import concourse.bass as bass
import concourse.tile as tile
from concourse import bacc, mybir
from concourse import bass_utils

F32 = mybir.dt.float32
F32R = mybir.dt.float32r
BF16 = mybir.dt.bfloat16
AF = mybir.ActivationFunctionType
ADD = mybir.AluOpType.add

# problem dims (hardcoded per contract)
B = 2
N = 2048
C = 2048
HEADS = 16
HD = 128          # head dim
NCORES = 8
HPC = HEADS // NCORES  # heads per core = 2
E = HPC * HD      # per-core projection width = 256
BN = B * N        # 4096
P = 128
CT = C // P       # 16 contraction tiles
NCH = 512         # n-chunk width for projections
NCHUNKS = BN // NCH   # 8
QCW = 512         # q-chunk width in attention
QCHUNKS = N // QCW    # 4 per batch
KT_PER_B = N // P     # 16 k-tiles per batch
TOK_TILES = BN // P   # 32
SCALE = float(HD) ** -0.5
XQ = 8            # x streamed in eighths of 2 c-tiles

_CACHE = {}


def _weave(*streams):
    """Emit step-closures from several streams, interleaved proportionally
    so all streams finish together. Each step is a 0-arg callable."""
    streams = [list(s) for s in streams if s]
    if not streams:
        return
    total = max(len(s) for s in streams)
    idx = [0.0] * len(streams)
    inc = [len(s) / total for s in streams]
    for _ in range(total):
        for i, s in enumerate(streams):
            idx[i] += inc[i]
            while idx[i] >= 1.0 and s:
                s.pop(0)()
                idx[i] -= 1.0
    for s in streams:
        for step in s:
            step()


def _build():
    nc = bacc.Bacc(
        "TRN2",
        target_bir_lowering=False,
        debug=False,
        enable_asserts=False,
        num_devices=NCORES,
    )

    xT = nc.dram_tensor("xT", [C, BN], BF16, kind="ExternalInput").ap()
    # wq/wk/wv packed per 2-c-tile piece: [piece, p, wi, t, e] so one DMA
    # brings the piece of all three projections
    wpk = nc.dram_tensor("wpk", [XQ, P, 3, 2, E], BF16,
                         kind="ExternalInput").ap()
    woT = nc.dram_tensor("woT", [E, C], BF16, kind="ExternalInput").ap()
    bqh = nc.dram_tensor("bqh", [HPC, P], F32, kind="ExternalInput").ap()
    bkh = nc.dram_tensor("bkh", [HPC, P], F32, kind="ExternalInput").ap()
    bvh = nc.dram_tensor("bvh", [HPC, P], F32, kind="ExternalInput").ap()
    masks = nc.dram_tensor("masks", [P, P], BF16, kind="ExternalInput").ap()
    ones_d = nc.dram_tensor("ones_d", [P, P], F32R, kind="ExternalInput").ap()
    yp = nc.dram_tensor("yp", [BN, C], BF16, kind="ExternalOutput").ap()

    with tile.TileContext(nc) as tc:
        with tc.tile_pool(name="persist", bufs=1) as persist, \
             tc.tile_pool(name="wpool", bufs=1) as wpool, \
             tc.tile_pool(name="xpool", bufs=24) as xpool, \
             tc.tile_pool(name="vtpool", bufs=2) as vtpool, \
             tc.tile_pool(name="epool", bufs=10) as epool, \
             tc.tile_pool(name="ctxpool", bufs=6) as ctxpool, \
             tc.tile_pool(name="espool", bufs=4) as espool, \
             tc.tile_pool(name="smpool", bufs=3) as smpool, \
             tc.tile_pool(name="ysbpool", bufs=3) as ysbpool, \
             tc.tile_pool(name="pacc_ps", bufs=2, space="PSUM") as pacc, \
             tc.tile_pool(name="sps_ps", bufs=2, space="PSUM") as spool, \
             tc.tile_pool(name="ctx_ps", bufs=2, space="PSUM") as cps, \
             tc.tile_pool(name="ysum_ps", bufs=2, space="PSUM") as yps:
            qT = persist.tile([P, HPC, B, N], BF16, tag="qT")
            kT = persist.tile([P, HPC, B, N], BF16, tag="kT")
            vN = persist.tile([P, TOK_TILES, E], BF16, tag="vN")
            masks_sb = persist.tile([P, P], BF16, tag="masks")
            ones_sb = persist.tile([P, P], F32R, tag="ones")

            wqkv_sb = wpool.tile([P, 3, CT, E], BF16, tag="wqkv")
            bq_sb = wpool.tile([P, HPC], F32, tag="bq")
            bk_sb = wpool.tile([P, HPC], F32, tag="bk")
            bv_sb = wpool.tile([P, HPC], F32, tag="bv")
            wo_sb = wpool.tile([P, HPC, C], BF16, tag="wo")

            wbias = [(0, bq_sb, bqh), (1, bk_sb, bkh), (2, bv_sb, bvh)]
            xTr = xT.rearrange("(t p) n -> p t n", p=P)

            def dma_x_chunk(ch):
                """Prefetch the 8 pieces of x chunk ch."""
                n0 = ch * NCH
                xh = []
                for piece in range(XQ):
                    xc = xpool.tile([P, CT // XQ, NCH], BF16, tag="xc",
                                    name=f"xc_{ch}_{piece}")
                    nc.sync.dma_start(
                        xc[:], xTr[:, piece * 2:(piece + 1) * 2,
                                   n0:n0 + NCH])
                    xh.append(xc)
                return xh

            # PE warm-up: memset a dummy tile and run throwaway matmuls so
            # the tensor engine leaves its low p-state while the first x/w
            # pieces are still in flight.
            warm = wpool.tile([P, P], BF16, tag="warm")
            warm_ps = pacc.tile([P, P], F32, tag="pacc", name="warm_ps")
            nc.gpsimd.memset(warm[:], 0.0)
            for _ in range(16):
                nc.tensor.matmul(warm_ps[:], warm[:], warm[:],
                                 start=True, stop=True)

            # Initial DMA priority: weight piece p and x piece p are
            # consumed together (2 c-tiles per piece), so interleave them,
            # one packed-w DMA + one x DMA per piece, all on the SP queue.
            xh0 = []
            wsbr = wqkv_sb[:].rearrange("p w (pc t) e -> p pc w t e", t=2)
            for piece in range(8):
                nc.sync.dma_start(wsbr[:, piece], wpk[piece])
                xc = xpool.tile([P, CT // XQ, NCH], BF16, tag="xc",
                                name=f"xc_0_{piece}")
                nc.sync.dma_start(
                    xc[:], xTr[:, piece * 2:(piece + 1) * 2, 0:NCH])
                xh0.append(xc)
                if piece == 2:
                    for (_, bdst, bsrc) in wbias:
                        nc.sync.dma_start(
                            bdst[:], bsrc.rearrange("h p -> p h"))
                    nc.sync.dma_start(masks_sb[:], masks)
                    nc.sync.dma_start(ones_sb[:], ones_d)
            nc.sync.dma_start(wo_sb[:], woT.rearrange("(h p) f -> p h f", p=P))

            # ---------------- step generators ----------------

            def proj_steps(ch, xh):
                """Projection of chunk ch: 6 runs (q/k/v x 2 heads) of 16
                accumulating matmuls, one PSUM bank at a time. Yields
                single-matmul closures; epilogue (bias copy / v transpose)
                rides with the last matmul of each run."""
                b = ch // (N // NCH)
                nn0 = (ch % (N // NCH)) * NCH
                steps = []
                for wi, bsb, _ in wbias:
                    for h in range(HPC):
                        state = {}

                        def mk(ct, bsb=bsb, wi=wi, h=h, state=state):
                            def step():
                                if ct == 0:
                                    state["acc"] = pacc.tile(
                                        [P, NCH], F32, tag="pacc",
                                        name=f"pacc_{ch}_{wi}_{h}")
                                nc.tensor.matmul(
                                    state["acc"][:],
                                    wqkv_sb[:, wi, ct, h * HD:(h + 1) * HD],
                                    xh[ct // 2][:, ct % 2, :],
                                    start=(ct == 0), stop=(ct == CT - 1),
                                )
                                if ct == CT - 1:
                                    acc = state["acc"]
                                    if wi < 2:
                                        dst = (qT if wi == 0 else kT)
                                        nc.scalar.activation(
                                            dst[:, h, b, nn0:nn0 + NCH],
                                            acc[:], AF.Identity,
                                            bias=bsb[:, h:h + 1], scale=1.0)
                                    else:
                                        vt = vtpool.tile(
                                            [P, NCH], BF16, tag="vt",
                                            name=f"vt_{ch}_{h}")
                                        nc.scalar.activation(
                                            vt[:], acc[:], AF.Identity,
                                            bias=bsb[:, h:h + 1], scale=1.0)
                                        nc.sync.dma_start(
                                            vN[:, ch * (NCH // P):
                                               (ch + 1) * (NCH // P),
                                               h * HD:(h + 1) * HD],
                                            vt[:], transpose=True)
                            return step
                        for ct in range(CT):
                            steps.append(mk(ct))
                return steps

            def attn_steps(b, qc, ctx_tiles):
                """Attention for block (b, qc): per (h, kt) one closure
                (S, exp, mask, esum, AV); per-h finalize closure appends
                the normalized ctx tile to ctx_tiles."""
                nkt = 4 * qc + 4
                steps = []
                for h in range(HPC):
                    state = {}

                    def emit_av(kt, state=state, h=h):
                        # AV for k-tile kt, one step behind its S/exp so
                        # the exp+mask chain is off the PE critical path
                        a = kt - 4 * qc
                        a0 = P * a if a > 0 else 0
                        nc.tensor.matmul(
                            state["ctxu"][:, a0:],
                            vN[:, b * KT_PER_B + kt,
                               h * HD:(h + 1) * HD],
                            state["et"][kt][:, a0:],
                            start=(kt == 0), stop=(kt == nkt - 1),
                        )

                    def mk(kt, h=h, state=state, emit_av=emit_av):
                        a = kt - 4 * qc
                        a0 = P * a if a > 0 else 0

                        def step():
                            if kt == 0:
                                state["ctxu"] = cps.tile(
                                    [P, QCW], F32, tag="ctxu",
                                    name=f"ctxu_{b}_{qc}_{h}")
                                state["esumA"] = espool.tile(
                                    [P, QCW], F32R, tag="esumA",
                                    name=f"esumA_{b}_{qc}_{h}")
                                state["esumB"] = espool.tile(
                                    [P, QCW], F32, tag="esumB",
                                    name=f"esumB_{b}_{qc}_{h}")
                                state["et"] = {}
                            esumA, esumB = state["esumA"], state["esumB"]
                            sps = spool.tile([P, QCW], F32, tag="s",
                                             name=f"s_{b}_{qc}_{h}_{kt}")
                            nc.tensor.matmul(
                                sps[:, a0:],
                                kT[:, h, b, kt * P:(kt + 1) * P],
                                qT[:, h, b, qc * QCW + a0:(qc + 1) * QCW],
                                start=True, stop=True,
                            )
                            et = epool.tile([P, QCW], BF16, tag="e",
                                            name=f"e_{b}_{qc}_{h}_{kt}")
                            state["et"][kt] = et
                            nc.scalar.activation(
                                et[:, a0:], sps[:, a0:], AF.Exp, scale=SCALE)
                            if a >= 0:  # diagonal: triangular [128,128]
                                nc.vector.tensor_mul(
                                    et[:, a0:a0 + P], et[:, a0:a0 + P],
                                    masks_sb[:])
                            # row-sum partials: GpSimd takes only every 4th
                            # k-tile (its per-op latency is ~2x DVE's, and
                            # its chain gates the sums matmul); DVE takes
                            # the rest. For qc>0, kt 0+1 fuse into one
                            # scalar_tensor_tensor (saves the init copy).
                            if kt == 0:
                                if qc == 0:
                                    nc.vector.tensor_copy(esumA[:], et[:])
                            elif kt == 1 and qc > 0:
                                nc.vector.scalar_tensor_tensor(
                                    esumA[:], state["et"][0][:], 1.0, et[:],
                                    op0=mybir.AluOpType.mult, op1=ADD)
                            elif kt == 3:
                                nc.gpsimd.tensor_copy(
                                    esumB[:, a0:], et[:, a0:])
                            elif kt % 4 == 3:
                                nc.gpsimd.tensor_tensor(
                                    esumB[:, a0:], esumB[:, a0:],
                                    et[:, a0:], op=ADD)
                            else:
                                nc.vector.tensor_tensor(
                                    esumA[:, a0:], esumA[:, a0:],
                                    et[:, a0:], op=ADD)
                            if kt >= 1:
                                emit_av(kt - 1)
                        return step

                    def fin(h=h, state=state, emit_av=emit_av):
                        def step():
                            emit_av(nkt - 1)
                            esumA, esumB = state["esumA"], state["esumB"]
                            # esumB covers k-tile 3's live range
                            c0 = 3 * P if qc == 0 else 0
                            nc.vector.tensor_tensor(
                                esumA[:, c0:], esumA[:, c0:], esumB[:, c0:],
                                op=ADD)
                            sums_bc = spool.tile([P, QCW], F32, tag="s",
                                                 name=f"sumbc_{b}_{qc}_{h}")
                            nc.tensor.matmul(
                                sums_bc[:], ones_sb[:], esumA[:],
                                start=True, stop=True)
                            recip_bc = smpool.tile([P, QCW], F32,
                                                   tag="recipbc",
                                                   name=f"recip_{b}_{qc}_{h}")
                            nc.vector.reciprocal_approx_fast(
                                recip_bc[:], sums_bc[:])
                            ctx = ctxpool.tile([P, QCW], BF16, tag="ctx",
                                               name=f"ctx_{b}_{qc}_{h}")
                            nc.vector.tensor_mul(
                                ctx[:], state["ctxu"][:], recip_bc[:])
                            ctx_tiles.append(ctx)
                        return step

                    for kt in range(nkt):
                        steps.append(mk(kt))
                    steps.append(fin())
                return steps

            def outproj_steps(b, qc, ctx_tiles):
                """Out-projection of block (b, qc): one closure per
                (nt, fc) = 2 accumulating matmuls + PSUM->SBUF copy; DMA
                per nt rides with the last fc."""
                steps = []
                for nt in range(QCW // P):
                    state = {}

                    def mk(fc, nt=nt, state=state):
                        def step():
                            if fc == 0:
                                state["ysb"] = ysbpool.tile(
                                    [P, C], BF16, tag="ysb",
                                    name=f"ysb_{b}_{qc}_{nt}")
                            y_ps = yps.tile([P, 512], F32, tag="yps",
                                            name=f"yps_{b}_{qc}_{nt}_{fc}")
                            for h in range(HPC):
                                nc.tensor.matmul(
                                    y_ps[:],
                                    ctx_tiles[h][:, nt * P:(nt + 1) * P],
                                    wo_sb[:, h, fc * 512:(fc + 1) * 512],
                                    start=(h == 0), stop=(h == HPC - 1),
                                )
                            y_sb = state["ysb"]
                            if fc % 2 == 0:
                                nc.vector.tensor_copy(
                                    y_sb[:, fc * 512:(fc + 1) * 512],
                                    y_ps[:])
                            else:
                                nc.scalar.copy(
                                    y_sb[:, fc * 512:(fc + 1) * 512],
                                    y_ps[:])
                            # write back per fc-pair so the final gate at
                            # kernel end is one small slice, not [128, 2048]
                            if fc % 2 == 1:
                                row0 = b * N + qc * QCW + nt * P
                                nc.sync.dma_start(
                                    yp[row0:row0 + P,
                                       (fc - 1) * 512:(fc + 1) * 512],
                                    y_sb[:, (fc - 1) * 512:(fc + 1) * 512])
                        return step
                    for fc in range(C // 512):
                        steps.append(mk(fc))
                return steps

            # ---------------- fused schedule ----------------
            # segment ch: proj(ch) ⊗ attn(block ch-1) ⊗ outproj(block ch-2)
            blocks = [(ch // QCHUNKS, ch % QCHUNKS) for ch in range(NCHUNKS)]
            ctxs = {}      # block index -> ctx tile list
            chunk_x = {0: xh0}
            for ch in range(NCHUNKS):
                if ch + 1 < NCHUNKS:
                    chunk_x[ch + 1] = dma_x_chunk(ch + 1)
                ps = proj_steps(ch, chunk_x.pop(ch))
                streams = [ps]
                if ch >= 1:
                    bb, qq = blocks[ch - 1]
                    ctxs[ch - 1] = []
                    streams.append(attn_steps(bb, qq, ctxs[ch - 1]))
                if ch >= 2:
                    bb, qq = blocks[ch - 2]
                    streams.append(outproj_steps(bb, qq, ctxs[ch - 2]))
                _weave(*streams)
            # tail: attn of the last block ⊗ outproj of the second-to-last,
            # then the last block's outproj
            bb, qq = blocks[NCHUNKS - 1]
            ctxs[NCHUNKS - 1] = []
            _weave(attn_steps(bb, qq, ctxs[NCHUNKS - 1]),
                   outproj_steps(*blocks[NCHUNKS - 2], ctxs[NCHUNKS - 2]))
            for step in outproj_steps(bb, qq, ctxs[NCHUNKS - 1]):
                step()

    nc.compile()
    return nc


def _host_prep(x, wq, bq, wk, bk, wv, bv, wo):
    """Build the 8 per-core input maps."""
    x = np.asarray(x, dtype=np.float32)
    xT = np.ascontiguousarray(x.reshape(BN, C).T).astype(ml_dtypes.bfloat16)

    # triangular mask for the partial 128-col block of a diagonal k-tile
    kl = np.arange(P)[:, None]
    jj = np.arange(P)[None, :]
    m = (jj >= kl).astype(ml_dtypes.bfloat16)

    in_maps = []
    for c in range(NCORES):
        e0 = c * E
        # packed weights: wpk[piece, p, wi, t, e] = w{wi}[e0+e, (2piece+t)*P+p]
        ws = np.stack([np.asarray(wq)[e0:e0 + E, :].T,
                       np.asarray(wk)[e0:e0 + E, :].T,
                       np.asarray(wv)[e0:e0 + E, :].T])  # [3, C, E]
        wpk = np.ascontiguousarray(
            ws.reshape(3, XQ, 2, P, E).transpose(1, 3, 0, 2, 4)
        ).astype(ml_dtypes.bfloat16)
        in_maps.append({
            "xT": xT,
            "wpk": wpk,
            "woT": np.ascontiguousarray(np.asarray(wo)[:, e0:e0 + E].T).astype(ml_dtypes.bfloat16),
            "bqh": np.ascontiguousarray(
                np.asarray(bq)[e0:e0 + E].reshape(HPC, P)).astype(np.float32),
            "bkh": np.ascontiguousarray(
                np.asarray(bk)[e0:e0 + E].reshape(HPC, P)).astype(np.float32),
            "bvh": np.ascontiguousarray(
                np.asarray(bv)[e0:e0 + E].reshape(HPC, P)).astype(np.float32),
            "masks": m,
            "ones_d": np.ones((P, P), dtype=np.float32),
        })
    return in_maps


def _ensure_ntff_hook_module():
    """run_bass_kernel_spmd(trace=True) imports antenv.axon_hooks; provide a
    stub (hook=None -> tracing skipped gracefully) if the module is absent."""
    try:
        import antenv.axon_hooks  # noqa: F401
    except ImportError:
        import sys
        import types
        try:
            import antenv
        except ImportError:
            return
        mod = types.ModuleType("antenv.axon_hooks")
        state = {"hook": None}
        mod.set_axon_ntff_profile_hook = lambda h: state.__setitem__("hook", h)
        mod.get_axon_ntff_profile_hook = lambda: state["hook"]
        sys.modules["antenv.axon_hooks"] = mod
        antenv.axon_hooks = mod


def kernel(**inputs):
    _ensure_ntff_hook_module()
    if "nc" not in _CACHE:
        _CACHE["nc"] = _build()
    nc = _CACHE["nc"]

    in_maps = _host_prep(
        inputs["x"], inputs["wq"], inputs["bq"], inputs["wk"], inputs["bk"],
        inputs["wv"], inputs["bv"], inputs["wo"],
    )

    res = bass_utils.run_bass_kernel_spmd(
        nc, in_maps, core_ids=list(range(NCORES)),
        trace=bool(os.environ.get("BASS_TRACE")),
    )
    _CACHE["last_result"] = res

    y = np.zeros((BN, C), dtype=np.float64)
    for c in range(NCORES):
        y += res.results[c]["yp"].astype(np.float64)
    y += np.asarray(inputs["bo"], dtype=np.float64)
    return y.astype(np.float32).reshape(B, N, C)
